# revision 31
# baseline (speedup 1.0000x reference)
"""CartBonded whole-pose scoring on 8 Trainium2 NeuronCores.

Sharding (pose-major, per sharding hint): core c owns poses [8c, 8c+8).
Host: buckets term lists by pose (stable sort), pads each (pose, type)
bucket to fixed [128, F] tiles, expands per-term spring constants
K = global_params[param_idx], and materializes per-term atom coords in
tile layout (the multi-index indirect-DMA path mis-orders indices on
TRN2 HW; per-element descriptor gather is far below HBM rate anyway, so
the gather rides the same host permutation that shards the term lists).
All device inputs ship as bf16 in component-planar layout [P, 3, F]
with Q poses packed per tile, halving HBM traffic and enabling the
DVE's 2x bf16 perf mode (contiguous step-1, 4B-aligned slices).
Device: DVE does the tensor-tensor math in bf16 2x mode; the scalar
engine (ACT) takes all unary work (squares, sqrt, trig, dtype casts)
plus the per-pose segment sums via activation accum_out, so the two
engines run in parallel. GPSIMD does no elementwise work: it shares
SBUF ports with the DVE and measured 4-8x DVE slowdowns when active.
Torsion uses the triple-product identity sin-term = -|b2|*(n1.b3) to
skip the m1 = n1 x b2n cross entirely, and fused custom DVE cubics
B(B^2-3A^2), A(3B^2-A^2) for the triple angle. Angle uses
theta = pi/2 - atan(x/y) (y > 0 always, so no quadrant fixup; the ACT
Arctan table saturates correctly for huge x/y).
Final cross-partition reduce via a ones-vector matmul on PE.
"""

import numpy as np
import ml_dtypes

N_POSES = 64
MAX_ATOMS = 16384
N_CORES = 8
PP = N_POSES // N_CORES  # poses per core
P = 128
PI = float(np.pi)

QB, QA, QT = 4, 4, 4  # poses per device tile, by type

_BUILD_CACHE = {}
_DVE_OPS = None


# ------------------------------------------------------------- custom DVE ops
def _register_dve_ops():
    global _DVE_OPS
    if _DVE_OPS is not None:
        return _DVE_OPS
    import concourse.dve_ops as dve_ops
    from concourse.dve_ops import DveOp, OPS, get_dve_sub_opcode
    from concourse.dve_spec import Spec, Src0, Src1, C0, C1, C2, sq, select, \
        lower, _has_src1
    from concourse.dve_uop import DveOpSpec

    def reg(name, spec):
        for existing in OPS:
            if existing.name == name:
                return existing
        op = DveOp(name, spec, subdim=False, uops_sha={})
        OPS.append(op)
        dve_ops._SUB_OPCODE_FOR_NAME[name] = (
            dve_ops._CUSTOM_DVE_ROW_BASE + len(OPS) - 1)
        for ver in ("v3", "v4"):
            c = DveOpSpec(name=name, opcode=get_dve_sub_opcode(name),
                          uops=lower(spec, ver=ver), rd1_en=_has_src1(spec))
            op.uops_sha[ver] = c.sha(ver)
        return op

    f32 = np.float32
    sqsum = reg("ANT_CB_SQSUM", Spec(
        body=sq(Src0) + sq(Src1) + C0,
        reference=lambda in0, in1, s0, s1, imm2:
            in0.astype(f32) ** 2 + in1.astype(f32) ** 2 + s0,
    ))
    cubic = reg("ANT_CB_CUBIC", Spec(
        body=Src0 * (sq(Src0) - C0 * sq(Src1)),
        reference=lambda in0, in1, s0, s1, imm2:
            in0.astype(f32) * (in0.astype(f32) ** 2
                               - s0 * in1.astype(f32) ** 2),
    ))
    _t = C0 - Src0
    rangew = reg("ANT_CB_RANGEW", Spec(
        body=select(_t < C1, _t + C2, _t),
        reference=lambda in0, in1, s0, s1, imm2:
            np.where((s0 - in0.astype(f32)) < s1,
                     (s0 - in0.astype(f32)) + imm2, s0 - in0.astype(f32)),
    ))
    _DVE_OPS = dict(sqsum=sqsum, cubic=cubic, rangew=rangew)
    return _DVE_OPS


# ----------------------------------------------------------------- host prep
def _prep_type(atoms, param_idx, x0, K_table, arity):
    """Bucket terms by pose, pad to [P, F] tiles.

    Returns F, idx [N_POSES, arity, P, F] int64 (global flat atom row),
    K [N_POSES, P, F] f32 (0 on pads), x0 [N_POSES, P, F] f32.
    """
    n = atoms.shape[0]
    pose = (atoms[:, 0] // MAX_ATOMS).astype(np.int64)
    order = np.argsort(pose, kind="stable")
    pose_s = pose[order]
    atoms_s = atoms[order].astype(np.int64)
    x0_s = x0[order]
    K_s = K_table[param_idx[order]]

    counts = np.bincount(pose, minlength=N_POSES)
    F = -(-int(counts.max()) // P)  # ceil(max/P)
    F = -(-F // 8) * 8  # multiple of 8
    starts = np.zeros(N_POSES + 1, np.int64)
    np.cumsum(counts, out=starts[1:])
    r = np.arange(n, dtype=np.int64) - starts[pose_s]
    part = (r // F).astype(np.int64)
    free = (r % F).astype(np.int64)
    assert part.max() < P

    idx = np.zeros((N_POSES, arity, P, F), np.int64)
    idx[pose_s, :, part, free] = atoms_s
    Kp = np.zeros((N_POSES, P, F), np.float32)
    Kp[pose_s, part, free] = K_s
    x0p = np.zeros((N_POSES, P, F), np.float32)
    x0p[pose_s, part, free] = x0_s
    return F, idx, Kp, x0p


def _pack_g(flat_bf16, idx, arity, Q, F):
    """Gather + reorder to [N_CORES, PP//Q, arity, P, 3, Q*F] bf16."""
    g = flat_bf16[idx]  # [N_POSES, arity, P, F, 3]
    g = g.reshape(N_CORES, PP // Q, Q, arity, P, F, 3)
    g = np.ascontiguousarray(g.transpose(0, 1, 3, 4, 6, 2, 5))
    return g.reshape(N_CORES, PP // Q, arity, P, 3, Q * F)


def _pack_s(a, Q, F):
    """[N_POSES, P, F] f32 -> [N_CORES, PP//Q, P, Q*F] bf16."""
    a = a.reshape(N_CORES, PP // Q, Q, P, F)
    a = np.ascontiguousarray(a.transpose(0, 1, 3, 2, 4))
    return a.reshape(N_CORES, PP // Q, P, Q * F).astype(ml_dtypes.bfloat16)


# --------------------------------------------------------------- device build
def _build(Fb, Fa, Ft):
    key = (Fb, Fa, Ft)
    if key in _BUILD_CACHE:
        return _BUILD_CACHE[key]

    ops = _register_dve_ops()

    import concourse.tile as tile
    from concourse import bacc, mybir

    dt = mybir.dt
    f32, bf16 = dt.float32, dt.bfloat16
    Act = mybir.ActivationFunctionType
    Op = mybir.AluOpType

    nc = bacc.Bacc("TRN2", target_bir_lowering=False, debug=False,
                   num_devices=N_CORES)

    NTB, NTA, NTT = PP // QB, PP // QA, PP // QT
    FB, FA, FT = QB * Fb, QA * Fa, QT * Ft

    bg = nc.dram_tensor("bg", [NTB, 2, P, 3, FB], bf16,
                        kind="ExternalInput").ap()
    bK = nc.dram_tensor("bK", [NTB, P, FB], bf16, kind="ExternalInput").ap()
    bx0 = nc.dram_tensor("bx0", [NTB, P, FB], bf16, kind="ExternalInput").ap()
    ag = nc.dram_tensor("ag", [NTA, 3, P, 3, FA], bf16,
                        kind="ExternalInput").ap()
    aK = nc.dram_tensor("aK", [NTA, P, FA], bf16, kind="ExternalInput").ap()
    ax0 = nc.dram_tensor("ax0", [NTA, P, FA], bf16, kind="ExternalInput").ap()
    tg = nc.dram_tensor("tg", [NTT, 4, P, 3, FT], bf16,
                        kind="ExternalInput").ap()
    tK = nc.dram_tensor("tK", [NTT, P, FT], bf16, kind="ExternalInput").ap()
    tx0 = nc.dram_tensor("tx0", [NTT, P, FT], bf16, kind="ExternalInput").ap()
    out = nc.dram_tensor("out", [1, PP], f32, kind="ExternalOutput").ap()

    for v in (0.0, 1e-12, -PI, -PI / 2):
        t = nc.alloc_sbuf_tensor(f"constf32-{v}", [P, 1], f32)
        nc.gpsimd.memset(t.ap(), v)
        nc.const_aps.aps[(f32, v)] = t.ap()
    nc.all_engine_barrier()

    from contextlib import ExitStack

    with tile.TileContext(nc) as tc, ExitStack() as ctx:
        pers = ctx.enter_context(tc.tile_pool(name="pers", bufs=1))
        gpool = ctx.enter_context(tc.tile_pool(name="g", bufs=2))
        xkpool = ctx.enter_context(tc.tile_pool(name="xk", bufs=2))
        tp = ctx.enter_context(tc.tile_pool(name="tmp", bufs=1))
        psum = ctx.enter_context(tc.tile_pool(name="ps", bufs=1, space="PSUM"))

        # partials column layout: pose*4 + {0: bond, 1: angle, 2: torsion
        # K*cos term, 3: torsion K sum (the "+1" in K*(1+cos))}
        partials = pers.tile([P, PP * 4], f32)

        V = nc.vector
        A = nc.scalar

        def gather(g_dram, ti, slot, F):
            g = gpool.tile([P, 3, F], bf16, tag=f"g{slot}", name=f"g{slot}")
            nc.sync.dma_start(g[:], g_dram[ti, slot])
            return g

        def loadxk(K_dram, x0_dram, ti, F):
            K = xkpool.tile([P, F], bf16, tag="K", name="Kt")
            nc.sync.dma_start(K[:], K_dram[ti])
            X0 = xkpool.tile([P, F], bf16, tag="X0", name="X0t")
            nc.sync.dma_start(X0[:], x0_dram[ti])
            return K, X0

        # shared scratch tags (sized by the largest type so all reuse SBUF)
        def V3(i, F):
            return tp.tile([P, 3, F], bf16, tag=f"v3_{i}", name=f"v3_{i}")

        def S(i, F):
            return tp.tile([P, F], bf16, tag=f"s_{i}", name=f"s_{i}")

        def W(i, F):
            return tp.tile([P, F], f32, tag=f"w_{i}", name=f"w_{i}")

        def vsub(o, a, b):
            V.tensor_tensor(out=o, in0=a, in1=b, op=Op.subtract)

        def vmul(o, a, b):
            V.tensor_tensor(out=o, in0=a, in1=b, op=Op.mult)

        def vadd(o, a, b):
            V.tensor_tensor(out=o, in0=a, in1=b, op=Op.add)

        def vdiff(i, gA, gB):
            o = V3(i, gA.shape[2])
            vsub(o[:], gA[:], gB[:])
            return o

        def cross(dst_i, u, v, F):
            o = V3(dst_i, F)
            for c in range(3):
                ta = S(8, F)
                vmul(ta[:], u[:, (c + 1) % 3, :], v[:, (c + 2) % 3, :])
                tb = S(9, F)
                vmul(tb[:], u[:, (c + 2) % 3, :], v[:, (c + 1) % 3, :])
                vsub(o[:, c, :], ta[:], tb[:])
            return o

        def DM(F):
            # dot/norm2 plane scratch; bufs=2 so consecutive dots rotate
            return tp.tile([P, 3, F], bf16, tag="dm", name="dm", bufs=2)

        def QS(F):
            return tp.tile([P, 3, F], bf16, tag="qs", name="qs", bufs=2)

        def fold3(m, dst_i, F):
            acc = S(dst_i, F)
            vadd(acc[:], m[:, 0, :], m[:, 1, :])
            vadd(acc[:], acc[:], m[:, 2, :])
            return acc[:]

        def dot(u, v, F, dst_i):
            m = DM(F)
            vmul(m[:], u[:], v[:])
            return fold3(m, dst_i, F)

        def norm2(u, F, dst_i):
            # squares on ACT (one pass over all 3 planes), folds on DVE
            qs = QS(F)
            A.activation(qs[:], u[:], Act.Square)
            return fold3(qs, dst_i, F)

        def accumulate(e, base_col, Q, Fq):
            # per-pose segment sums on ACT via activation accum_out
            for q in range(Q):
                sl = slice(q * Fq, (q + 1) * Fq)
                scr = S(10, Fq)
                col = base_col + 4 * q
                A.activation(scr[:], e[:, sl], Act.Identity,
                             accum_out=partials[:, col:col + 1])

        def norm2_dve(m_i, dst_i, u, F):
            # all-DVE variant for the kernel head, before ACT warms up
            m = V3(m_i, F)
            vmul(m[:], u[:], u[:])
            o = S(dst_i, F)
            vadd(o[:], m[:, 0, :], m[:, 1, :])
            vadd(o[:], o[:], m[:, 2, :])
            return o

        # ---------------- bond: K * (|r0 - r1| - x0)^2
        def bond(ti, tail=False):
            g0 = gather(bg, ti, 0, FB)
            g1 = gather(bg, ti, 1, FB)
            K, X0 = loadxk(bK, bx0, ti, FB)
            d = vdiff(0, g0, g1)
            D2 = norm2_dve(1, 0, d, FB)                  # s0
            dist = S(1, FB)
            A.activation(dist[:], D2[:], Act.Sqrt, bias=1e-12)
            dd = S(5, FB)
            vsub(dd[:], dist[:], X0[:])
            sq = S(3, FB)
            if tail:
                vmul(sq[:], dd[:], dd[:])
            else:
                A.activation(sq[:], dd[:], Act.Square)
            if tail:
                # last tile: segment sums on the (idle-by-now) DVE so the
                # kernel tail is not a serial ACT chain
                for q in range(QB):
                    sl = slice(q * Fb, (q + 1) * Fb)
                    e = S(4, Fb)
                    col = ti * QB * 4 + 0 + 4 * q
                    V.scalar_tensor_tensor(
                        out=e[:], in0=sq[:, sl], scalar=0.0, in1=K[:, sl],
                        op0=Op.add, op1=Op.mult,
                        accum_out=partials[:, col:col + 1])
            else:
                e = S(4, FB)
                vmul(e[:], sq[:], K[:])
                accumulate(e, ti * QB * 4 + 0, QB, Fb)

        # ------- angle: K * (theta - x0)^2, theta = pi/2 - atan(x/y), y>0
        # |uxv|^2 = |u|^2*|v|^2 - (u.v)^2 (Lagrange), clamped at 0 --
        # cheaper than the explicit cross product; the rare near-parallel
        # cancellation error is noise at the pose-sum level.
        def angle(ti):
            g0 = gather(ag, ti, 0, FA)
            g1 = gather(ag, ti, 1, FA)
            g2 = gather(ag, ti, 2, FA)
            K, X0 = loadxk(aK, ax0, ti, FA)
            xc = S(7, FA)
            A.activation(xc[:], X0[:], Act.Identity, bias=-PI / 2)
            u = vdiff(0, g0, g1)
            v = vdiff(1, g2, g1)
            # issue both squares on ACT before the x-dot occupies the DVE,
            # so the uu/vv folds find their inputs ready
            qs_u = QS(FA)
            A.activation(qs_u[:], u[:], Act.Square)
            qs_v = QS(FA)
            A.activation(qs_v[:], v[:], Act.Square)
            x = dot(u, v, FA, 0)
            uu = fold3(qs_u, 1, FA)
            vv = fold3(qs_v, 2, FA)
            x2 = S(3, FA)
            vmul(x2[:], x, x)
            m = S(4, FA)
            vmul(m[:], uu, vv)
            Sn = S(5, FA)
            vsub(Sn[:], m[:], x2[:])
            Sc = S(6, FA)
            A.activation(Sc[:], Sn[:], Act.Relu)
            y = W(0, FA)
            A.activation(y[:], Sc[:], Act.Sqrt, bias=1e-12)
            yi = W(1, FA)
            V.reciprocal_approx_fast(yi[:], y[:])
            t = S(2, FA)
            vmul(t[:], x, yi[:])
            phi = S(3, FA)
            A.activation(phi[:], t[:], Act.Arctan)
            dd = S(4, FA)
            vadd(dd[:], phi[:], xc[:])
            sq = S(5, FA)
            A.activation(sq[:], dd[:], Act.Square)
            e = S(6, FA)
            vmul(e[:], sq[:], K[:])
            accumulate(e, ti * QA * 4 + 1, QA, Fa)

        # ---------------- torsion: K * (1 + cos(3*phi - x0))
        def torsion(ti, defer_accum=False):
            g0 = gather(tg, ti, 0, FT)
            g1 = gather(tg, ti, 1, FT)
            g2 = gather(tg, ti, 2, FT)
            g3 = gather(tg, ti, 3, FT)
            K, X0 = loadxk(tK, tx0, ti, FT)
            # K-sums depend only on the K DMA: accumulate right away so
            # they never sit on the kernel tail
            accumulate(K, ti * QT * 4 + 3, QT, Ft)
            # x0 trig first: only needs X0, keeps ACT busy during crosses.
            # cos(x0) = 1 - 2*sin(x0/2)^2 keeps the Sin arg inside [0, pi);
            # the cheap DVE combine is emitted LATE so the in-order DVE
            # queue never waits on this ACT chain.
            sh = S(5, FT)
            A.activation(sh[:], X0[:], Act.Sin, scale=0.5)
            sh2 = S(6, FT)
            A.activation(sh2[:], sh[:], Act.Square)
            sx0n = S(7, FT)
            A.activation(sx0n[:], X0[:], Act.Sin, bias=-PI)
            b1 = vdiff(0, g1, g0)
            b2 = vdiff(1, g2, g1)
            b3 = vdiff(2, g3, g2)
            n1 = cross(3, b1, b2, FT)                    # b1 dead, v3_0 free
            n2 = cross(4, b2, b3, FT)
            S2 = norm2(b2, FT, 1)
            r = S(0, FT)
            A.activation(r[:], S2, Act.Sqrt, bias=1e-12)
            d13 = dot(n1, b3, FT, 2)
            B = dot(n1, n2, FT, 3)
            # ref sin-term A = (n1 x b2_hat).n2 = -|b2|*(n1.b3); carry
            # Ap = -A and flip the sin sign downstream.
            Ap = S(1, FT)
            vmul(Ap[:], r[:], d13)
            R2 = W(0, FT)
            V._custom_dve(ops["sqsum"], out=R2[:], in0=Ap[:], in1=B,
                          s0=1e-6)
            r2i = W(1, FT)
            V.reciprocal_approx_fast(r2i[:], R2[:])
            iR = S(2, FT)                                # d13 dead
            A.activation(iR[:], r2i[:], Act.Sqrt)
            iR3 = S(4, FT)
            vmul(iR3[:], r2i[:], iR[:])
            # R^3*cos(3phi) = B(B^2-3A^2);  R^3*sin(3phi) = A(3B^2-A^2)
            T1 = S(0, FT)                                # r dead
            V._custom_dve(ops["cubic"], out=T1[:], in0=B, in1=Ap[:],
                          s0=3.0)
            T2p = S(2, FT)                               # = -R^3 sin(3phi)
            V._custom_dve(ops["cubic"], out=T2p[:], in0=Ap[:], in1=B,
                          s0=3.0)
            # cos(3phi - x0) = [T1*cx0 + T2p*sx0n] * iR^3
            cx0 = S(8, FT)
            V.tensor_scalar(out=cx0[:], in0=sh2[:], scalar1=-2.0,
                            scalar2=1.0, op0=Op.mult, op1=Op.add)
            w = S(1, FT)                                 # Ap dead
            vmul(w[:], T1[:], cx0[:])
            vv = S(3, FT)                                # B dead
            vmul(vv[:], T2p[:], sx0n[:])
            uu = S(0, FT)
            vadd(uu[:], w[:], vv[:])
            gq = S(1, FT)
            vmul(gq[:], uu[:], iR3[:])
            # K*(1 + cos) = K + K*cos: accumulate K*cos and K separately
            # (the K sums ride the idle ACT engine, saving a DVE pass)
            e = S(2, FT)
            vmul(e[:], gq[:], K[:])
            if defer_accum:
                return (e, ti * QT * 4 + 2)
            accumulate(e, ti * QT * 4 + 2, QT, Ft)
            return None

        # order: a small bond tile first primes the pipeline (least DMA
        # before first compute); the other bond tile LAST gives the kernel
        # a short serial tail instead of torsion's deep one.
        # warm the sqrt activation-table during the DMA lead-in so
        # bond0's dist Sqrt doesn't pay the 1.28us table load in-line
        warm = pers.tile([P, 1], f32)
        A.activation(warm[:], nc.const_aps.aps[(0, 0.0) if False else (f32, 0.0)][:P],
                     Act.Sqrt)
        bond(0)
        deferred = None
        for ti in range(max(NTA, NTT)):
            if ti < NTA:
                angle(ti)
            if ti < NTT:
                deferred = torsion(ti, defer_accum=(ti == NTT - 1))
        # the last torsion tile's ACT accums are emitted after the tail
        # bond so they overlap its DVE work instead of serializing ACT
        bond(1, tail=True)
        if deferred is not None:
            accumulate(deferred[0], deferred[1], QT, Ft)

        ones = pers.tile([P, 1], f32)
        V.memset(ones[:], 1.0)
        ps = psum.tile([1, PP * 4], f32)
        nc.tensor.matmul(out=ps[:], lhsT=ones[:], rhs=partials[:],
                         start=True, stop=True)
        psc = pers.tile([1, PP * 4], f32)
        V.tensor_copy(out=psc[:], in_=ps[:])
        s8 = pers.tile([1, PP], f32)
        V.tensor_tensor(out=s8[:], in0=psc[0:1, 0:PP * 4:4],
                        in1=psc[0:1, 1:PP * 4:4], op=Op.add)
        V.tensor_tensor(out=s8[:], in0=s8[:], in1=psc[0:1, 2:PP * 4:4],
                        op=Op.add)
        V.tensor_tensor(out=s8[:], in0=s8[:], in1=psc[0:1, 3:PP * 4:4],
                        op=Op.add)
        nc.sync.dma_start(out[:], s8[:])

    nc.compile()
    _BUILD_CACHE[key] = nc
    return nc


# ---------------------------------------------------------------------- main
def kernel(coords, global_params, bond_x0, angle_x0, tor_x0,
           bond_atoms, bond_param_idx, angle_atoms, angle_param_idx,
           tor_atoms, tor_param_idx, _trace=False):
    coords = np.asarray(coords, dtype=np.float32)
    K_table = np.asarray(global_params, dtype=np.float32)[:, 0]

    Fb, bidx, bKp, bx0p = _prep_type(np.asarray(bond_atoms),
                                     np.asarray(bond_param_idx),
                                     np.asarray(bond_x0, np.float32),
                                     K_table, 2)
    Fa, aidx, aKp, ax0p = _prep_type(np.asarray(angle_atoms),
                                     np.asarray(angle_param_idx),
                                     np.asarray(angle_x0, np.float32),
                                     K_table, 3)
    Ft, tidx, tKp, tx0p = _prep_type(np.asarray(tor_atoms),
                                     np.asarray(tor_param_idx),
                                     np.asarray(tor_x0, np.float32),
                                     K_table, 4)

    nc = _build(Fb, Fa, Ft)

    flat = coords.reshape(-1, 3).astype(ml_dtypes.bfloat16)

    in_maps = []
    bgq = _pack_g(flat, bidx, 2, QB, Fb)
    agq = _pack_g(flat, aidx, 3, QA, Fa)
    tgq = _pack_g(flat, tidx, 4, QT, Ft)
    bKq, bx0q = _pack_s(bKp, QB, Fb), _pack_s(bx0p, QB, Fb)
    aKq, ax0q = _pack_s(aKp, QA, Fa), _pack_s(ax0p, QA, Fa)
    tKq, tx0q = _pack_s(tKp, QT, Ft), _pack_s(tx0p, QT, Ft)

    for c in range(N_CORES):
        in_maps.append({
            "bg": bgq[c], "bK": bKq[c], "bx0": bx0q[c],
            "ag": agq[c], "aK": aKq[c], "ax0": ax0q[c],
            "tg": tgq[c], "tK": tKq[c], "tx0": tx0q[c],
        })

    from concourse.bass_utils import run_bass_kernel_spmd
    res = run_bass_kernel_spmd(nc, in_maps, list(range(N_CORES)),
                               trace=_trace)
    out = np.concatenate([res.results[c]["out"][0] for c in range(N_CORES)])
    if _trace:
        kernel._last_result = res
    return out.astype(np.float32)


# revision 33
# speedup vs baseline: 1.1792x; 1.1792x over previous
"""CartBonded whole-pose scoring on 8 Trainium2 NeuronCores.

Sharding (pose-major, per sharding hint): core c owns poses [8c, 8c+8).
Host: buckets term lists by pose (stable sort), pads each (pose, type)
bucket to fixed [128, F] tiles, expands per-term spring constants
K = global_params[param_idx], and materializes per-term atom coords in
tile layout (the multi-index indirect-DMA path mis-orders indices on
TRN2 HW; per-element descriptor gather is far below HBM rate anyway, so
the gather rides the same host permutation that shards the term lists).
All device inputs ship as bf16 in component-planar layout [P, 3, F]
with Q poses packed per tile, halving HBM traffic and enabling the
DVE's 2x bf16 perf mode (contiguous step-1, 4B-aligned slices).
Device: DVE does the tensor-tensor math in bf16 2x mode; the scalar
engine (ACT) takes all unary work (squares, sqrt, trig, dtype casts)
plus the per-pose segment sums via activation accum_out, so the two
engines run in parallel. GPSIMD does no elementwise work: it shares
SBUF ports with the DVE and measured 4-8x DVE slowdowns when active.
Torsion uses the triple-product identity sin-term = -|b2|*(n1.b3) to
skip the m1 = n1 x b2n cross entirely, and fused custom DVE cubics
B(B^2-3A^2), A(3B^2-A^2) for the triple angle. Angle uses
theta = pi/2 - atan(x/y) (y > 0 always, so no quadrant fixup; the ACT
Arctan table saturates correctly for huge x/y).
Final cross-partition reduce via a ones-vector matmul on PE.
"""

import numpy as np
import ml_dtypes

N_POSES = 64
MAX_ATOMS = 16384
N_CORES = 8
PP = N_POSES // N_CORES  # poses per core
P = 128
PI = float(np.pi)

QB, QA, QT = 4, 4, 4  # poses per device tile, by type

_BUILD_CACHE = {}
_DVE_OPS = None


# ------------------------------------------------------------- custom DVE ops
def _register_dve_ops():
    global _DVE_OPS
    if _DVE_OPS is not None:
        return _DVE_OPS
    import concourse.dve_ops as dve_ops
    from concourse.dve_ops import DveOp, OPS, get_dve_sub_opcode
    from concourse.dve_spec import Spec, Src0, Src1, C0, C1, C2, sq, select, \
        lower, _has_src1
    from concourse.dve_uop import DveOpSpec

    def reg(name, spec):
        for existing in OPS:
            if existing.name == name:
                return existing
        op = DveOp(name, spec, subdim=False, uops_sha={})
        OPS.append(op)
        dve_ops._SUB_OPCODE_FOR_NAME[name] = (
            dve_ops._CUSTOM_DVE_ROW_BASE + len(OPS) - 1)
        for ver in ("v3", "v4"):
            c = DveOpSpec(name=name, opcode=get_dve_sub_opcode(name),
                          uops=lower(spec, ver=ver), rd1_en=_has_src1(spec))
            op.uops_sha[ver] = c.sha(ver)
        return op

    f32 = np.float32
    sqsum = reg("ANT_CB_SQSUM", Spec(
        body=sq(Src0) + sq(Src1) + C0,
        reference=lambda in0, in1, s0, s1, imm2:
            in0.astype(f32) ** 2 + in1.astype(f32) ** 2 + s0,
    ))
    cubic = reg("ANT_CB_CUBIC", Spec(
        body=Src0 * (sq(Src0) - C0 * sq(Src1)),
        reference=lambda in0, in1, s0, s1, imm2:
            in0.astype(f32) * (in0.astype(f32) ** 2
                               - s0 * in1.astype(f32) ** 2),
    ))
    _t = C0 - Src0
    rangew = reg("ANT_CB_RANGEW", Spec(
        body=select(_t < C1, _t + C2, _t),
        reference=lambda in0, in1, s0, s1, imm2:
            np.where((s0 - in0.astype(f32)) < s1,
                     (s0 - in0.astype(f32)) + imm2, s0 - in0.astype(f32)),
    ))
    _DVE_OPS = dict(sqsum=sqsum, cubic=cubic, rangew=rangew)
    return _DVE_OPS


# ----------------------------------------------------------------- host prep
def _prep_type(atoms, param_idx, x0, K_table, arity):
    """Bucket terms by pose, pad to [P, F] tiles.

    Returns F, idx [N_POSES, arity, P, F] int64 (global flat atom row),
    K [N_POSES, P, F] f32 (0 on pads), x0 [N_POSES, P, F] f32.
    """
    n = atoms.shape[0]
    pose = (atoms[:, 0] // MAX_ATOMS).astype(np.int64)
    order = np.argsort(pose, kind="stable")
    pose_s = pose[order]
    atoms_s = atoms[order].astype(np.int64)
    x0_s = x0[order]
    K_s = K_table[param_idx[order]]

    counts = np.bincount(pose, minlength=N_POSES)
    F = -(-int(counts.max()) // P)  # ceil(max/P)
    F = -(-F // 8) * 8  # multiple of 8
    starts = np.zeros(N_POSES + 1, np.int64)
    np.cumsum(counts, out=starts[1:])
    r = np.arange(n, dtype=np.int64) - starts[pose_s]
    part = (r // F).astype(np.int64)
    free = (r % F).astype(np.int64)
    assert part.max() < P

    idx = np.zeros((N_POSES, arity, P, F), np.int64)
    idx[pose_s, :, part, free] = atoms_s
    Kp = np.zeros((N_POSES, P, F), np.float32)
    Kp[pose_s, part, free] = K_s
    x0p = np.zeros((N_POSES, P, F), np.float32)
    x0p[pose_s, part, free] = x0_s
    return F, idx, Kp, x0p


def _pack_g(flat_bf16, idx, arity, Q, F):
    """Gather + reorder to [N_CORES, PP//Q, arity, P, 3, Q*F] bf16."""
    g = flat_bf16[idx]  # [N_POSES, arity, P, F, 3]
    g = g.reshape(N_CORES, PP // Q, Q, arity, P, F, 3)
    g = np.ascontiguousarray(g.transpose(0, 1, 3, 4, 6, 2, 5))
    return g.reshape(N_CORES, PP // Q, arity, P, 3, Q * F)


def _pack_s(a, Q, F):
    """[N_POSES, P, F] f32 -> [N_CORES, PP//Q, P, Q*F] bf16."""
    a = a.reshape(N_CORES, PP // Q, Q, P, F)
    a = np.ascontiguousarray(a.transpose(0, 1, 3, 2, 4))
    return a.reshape(N_CORES, PP // Q, P, Q * F).astype(ml_dtypes.bfloat16)


# --------------------------------------------------------------- device build
def _build(Fb, Fa, Ft):
    key = (Fb, Fa, Ft)
    if key in _BUILD_CACHE:
        return _BUILD_CACHE[key]

    ops = _register_dve_ops()

    import concourse.tile as tile
    from concourse import bacc, mybir

    dt = mybir.dt
    f32, bf16 = dt.float32, dt.bfloat16
    Act = mybir.ActivationFunctionType
    Op = mybir.AluOpType

    nc = bacc.Bacc("TRN2", target_bir_lowering=False, debug=False,
                   num_devices=N_CORES)

    NTB, NTA, NTT = PP // QB, PP // QA, PP // QT
    FB, FA, FT = QB * Fb, QA * Fa, QT * Ft

    bg = nc.dram_tensor("bg", [NTB, 2, P, 3, FB], bf16,
                        kind="ExternalInput").ap()
    bK = nc.dram_tensor("bK", [NTB, P, FB], bf16, kind="ExternalInput").ap()
    bx0 = nc.dram_tensor("bx0", [NTB, P, FB], bf16, kind="ExternalInput").ap()
    ag = nc.dram_tensor("ag", [NTA, 3, P, 3, FA], bf16,
                        kind="ExternalInput").ap()
    aK = nc.dram_tensor("aK", [NTA, P, FA], bf16, kind="ExternalInput").ap()
    ax0 = nc.dram_tensor("ax0", [NTA, P, FA], bf16, kind="ExternalInput").ap()
    tg = nc.dram_tensor("tg", [NTT, 4, P, 3, FT], bf16,
                        kind="ExternalInput").ap()
    tK = nc.dram_tensor("tK", [NTT, P, FT], bf16, kind="ExternalInput").ap()
    tx0 = nc.dram_tensor("tx0", [NTT, P, FT], bf16, kind="ExternalInput").ap()
    out = nc.dram_tensor("out", [1, PP], f32, kind="ExternalOutput").ap()

    for v in (0.0, 1e-12, -PI, -PI / 2):
        t = nc.alloc_sbuf_tensor(f"constf32-{v}", [P, 1], f32)
        nc.gpsimd.memset(t.ap(), v)
        nc.const_aps.aps[(f32, v)] = t.ap()
    nc.all_engine_barrier()

    from contextlib import ExitStack

    with tile.TileContext(nc) as tc, ExitStack() as ctx:
        pers = ctx.enter_context(tc.tile_pool(name="pers", bufs=1))
        gpool = ctx.enter_context(tc.tile_pool(name="g", bufs=2))
        xkpool = ctx.enter_context(tc.tile_pool(name="xk", bufs=2))
        tp = ctx.enter_context(tc.tile_pool(name="tmp", bufs=1))
        psum = ctx.enter_context(tc.tile_pool(name="ps", bufs=1, space="PSUM"))

        # partials column layout: pose*4 + {0: bond, 1: angle, 2: torsion
        # K*cos term, 3: torsion K sum (the "+1" in K*(1+cos))}
        partials = pers.tile([P, PP * 4], f32)

        V = nc.vector
        A = nc.scalar

        def gather(g_dram, ti, slot, F):
            g = gpool.tile([P, 3, F], bf16, tag=f"g{slot}", name=f"g{slot}")
            nc.sync.dma_start(g[:], g_dram[ti, slot])
            return g

        def loadxk(K_dram, x0_dram, ti, F):
            K = xkpool.tile([P, F], bf16, tag="K", name="Kt")
            nc.sync.dma_start(K[:], K_dram[ti])
            X0 = xkpool.tile([P, F], bf16, tag="X0", name="X0t")
            nc.sync.dma_start(X0[:], x0_dram[ti])
            return K, X0

        # shared scratch tags (sized by the largest type so all reuse SBUF)
        def V3(i, F):
            return tp.tile([P, 3, F], bf16, tag=f"v3_{i}", name=f"v3_{i}")

        def S(i, F):
            return tp.tile([P, F], bf16, tag=f"s_{i}", name=f"s_{i}")

        def W(i, F):
            return tp.tile([P, F], f32, tag=f"w_{i}", name=f"w_{i}")

        def vsub(o, a, b):
            V.tensor_tensor(out=o, in0=a, in1=b, op=Op.subtract)

        def vmul(o, a, b):
            V.tensor_tensor(out=o, in0=a, in1=b, op=Op.mult)

        def vadd(o, a, b):
            V.tensor_tensor(out=o, in0=a, in1=b, op=Op.add)

        def vdiff(i, gA, gB):
            o = V3(i, gA.shape[2])
            vsub(o[:], gA[:], gB[:])
            return o

        def cross(dst_i, u, v, F):
            o = V3(dst_i, F)
            for c in range(3):
                ta = S(8, F)
                vmul(ta[:], u[:, (c + 1) % 3, :], v[:, (c + 2) % 3, :])
                tb = S(9, F)
                vmul(tb[:], u[:, (c + 2) % 3, :], v[:, (c + 1) % 3, :])
                vsub(o[:, c, :], ta[:], tb[:])
            return o

        def DM(F):
            # dot/norm2 plane scratch; bufs=2 so consecutive dots rotate
            return tp.tile([P, 3, F], bf16, tag="dm", name="dm", bufs=2)

        def QS(F):
            return tp.tile([P, 3, F], bf16, tag="qs", name="qs", bufs=2)

        def fold3(m, dst_i, F):
            acc = S(dst_i, F)
            vadd(acc[:], m[:, 0, :], m[:, 1, :])
            vadd(acc[:], acc[:], m[:, 2, :])
            return acc[:]

        def dot(u, v, F, dst_i):
            m = DM(F)
            vmul(m[:], u[:], v[:])
            return fold3(m, dst_i, F)

        def norm2(u, F, dst_i):
            # squares on ACT (one pass over all 3 planes), folds on DVE
            qs = QS(F)
            A.activation(qs[:], u[:], Act.Square)
            return fold3(qs, dst_i, F)

        def accumulate(e, base_col, Q, Fq):
            # per-pose segment sums on ACT via activation accum_out
            for q in range(Q):
                sl = slice(q * Fq, (q + 1) * Fq)
                scr = S(10, Fq)
                col = base_col + 4 * q
                A.activation(scr[:], e[:, sl], Act.Identity,
                             accum_out=partials[:, col:col + 1])

        def norm2_dve(m_i, dst_i, u, F):
            # all-DVE variant for the kernel head, before ACT warms up
            m = V3(m_i, F)
            vmul(m[:], u[:], u[:])
            o = S(dst_i, F)
            vadd(o[:], m[:, 0, :], m[:, 1, :])
            vadd(o[:], o[:], m[:, 2, :])
            return o

        # ---------------- bond: K * (|r0 - r1| - x0)^2
        def bond(ti, tail=False):
            g0 = gather(bg, ti, 0, FB)
            g1 = gather(bg, ti, 1, FB)
            K, X0 = loadxk(bK, bx0, ti, FB)
            d = vdiff(0, g0, g1)
            D2 = norm2_dve(1, 0, d, FB)                  # s0
            dist = S(1, FB)
            A.activation(dist[:], D2[:], Act.Sqrt, bias=1e-12)
            dd = S(5, FB)
            vsub(dd[:], dist[:], X0[:])
            sq = S(3, FB)
            if tail:
                vmul(sq[:], dd[:], dd[:])
            else:
                A.activation(sq[:], dd[:], Act.Square)
            if tail:
                # last tile: segment sums on the (idle-by-now) DVE so the
                # kernel tail is not a serial ACT chain
                for q in range(QB):
                    sl = slice(q * Fb, (q + 1) * Fb)
                    e = S(4, Fb)
                    col = ti * QB * 4 + 0 + 4 * q
                    V.scalar_tensor_tensor(
                        out=e[:], in0=sq[:, sl], scalar=0.0, in1=K[:, sl],
                        op0=Op.add, op1=Op.mult,
                        accum_out=partials[:, col:col + 1])
            else:
                e = S(4, FB)
                vmul(e[:], sq[:], K[:])
                accumulate(e, ti * QB * 4 + 0, QB, Fb)

        # ------- angle: K * (theta - x0)^2, theta = pi/2 - atan(x/y), y>0
        # |uxv|^2 = |u|^2*|v|^2 - (u.v)^2 (Lagrange), clamped at 0 --
        # cheaper than the explicit cross product; the rare near-parallel
        # cancellation error is noise at the pose-sum level.
        def angle(ti, defer_accum=False):
            g0 = gather(ag, ti, 0, FA)
            g1 = gather(ag, ti, 1, FA)
            g2 = gather(ag, ti, 2, FA)
            K, X0 = loadxk(aK, ax0, ti, FA)
            xc = S(7, FA)
            A.activation(xc[:], X0[:], Act.Identity, bias=-PI / 2)
            u = vdiff(0, g0, g1)
            v = vdiff(1, g2, g1)
            # issue both squares on ACT before the x-dot occupies the DVE,
            # so the uu/vv folds find their inputs ready
            qs_u = QS(FA)
            A.activation(qs_u[:], u[:], Act.Square)
            qs_v = QS(FA)
            A.activation(qs_v[:], v[:], Act.Square)
            x = dot(u, v, FA, 0)
            uu = fold3(qs_u, 1, FA)
            vv = fold3(qs_v, 2, FA)
            x2 = S(3, FA)
            vmul(x2[:], x, x)
            m = S(4, FA)
            vmul(m[:], uu, vv)
            Sn = S(5, FA)
            vsub(Sn[:], m[:], x2[:])
            Sc = S(6, FA)
            A.activation(Sc[:], Sn[:], Act.Relu)
            y = W(0, FA)
            A.activation(y[:], Sc[:], Act.Sqrt, bias=1e-12)
            yi = W(1, FA)
            V.reciprocal_approx_fast(yi[:], y[:])
            t = S(2, FA)
            vmul(t[:], x, yi[:])
            phi = S(3, FA)
            A.activation(phi[:], t[:], Act.Arctan)
            dd = S(4, FA)
            vadd(dd[:], phi[:], xc[:])
            sq = S(5, FA)
            A.activation(sq[:], dd[:], Act.Square)
            e = tp.tile([P, FA], bf16, tag="ae", name="ae")
            vmul(e[:], sq[:], K[:])
            if defer_accum:
                return (e, ti * QA * 4 + 1)
            accumulate(e, ti * QA * 4 + 1, QA, Fa)
            return None

        # ---------------- torsion: K * (1 + cos(3*phi - x0))
        def torsion(ti, defer_accum=False):
            g0 = gather(tg, ti, 0, FT)
            g1 = gather(tg, ti, 1, FT)
            g2 = gather(tg, ti, 2, FT)
            g3 = gather(tg, ti, 3, FT)
            K, X0 = loadxk(tK, tx0, ti, FT)
            # K-sums depend only on the K DMA: accumulate right away so
            # they never sit on the kernel tail
            accumulate(K, ti * QT * 4 + 3, QT, Ft)
            # x0 trig first: only needs X0, keeps ACT busy during crosses.
            # cos(x0) = 1 - 2*sin(x0/2)^2 keeps the Sin arg inside [0, pi);
            # the cheap DVE combine is emitted LATE so the in-order DVE
            # queue never waits on this ACT chain.
            sh = S(5, FT)
            A.activation(sh[:], X0[:], Act.Sin, scale=0.5)
            sh2 = S(6, FT)
            A.activation(sh2[:], sh[:], Act.Square)
            sx0n = S(7, FT)
            A.activation(sx0n[:], X0[:], Act.Sin, bias=-PI)
            b1 = vdiff(0, g1, g0)
            b2 = vdiff(1, g2, g1)
            b3 = vdiff(2, g3, g2)
            n1 = cross(3, b1, b2, FT)                    # b1 dead, v3_0 free
            n2 = cross(4, b2, b3, FT)
            S2 = norm2(b2, FT, 1)
            r = S(0, FT)
            A.activation(r[:], S2, Act.Sqrt, bias=1e-12)
            d13 = dot(n1, b3, FT, 2)
            B = dot(n1, n2, FT, 3)
            # ref sin-term A = (n1 x b2_hat).n2 = -|b2|*(n1.b3); carry
            # Ap = -A and flip the sin sign downstream.
            Ap = S(1, FT)
            vmul(Ap[:], r[:], d13)
            R2 = W(0, FT)
            V._custom_dve(ops["sqsum"], out=R2[:], in0=Ap[:], in1=B,
                          s0=1e-6)
            r2i = W(1, FT)
            V.reciprocal_approx_fast(r2i[:], R2[:])
            iR = S(2, FT)                                # d13 dead
            A.activation(iR[:], r2i[:], Act.Sqrt)
            iR3 = S(4, FT)
            vmul(iR3[:], r2i[:], iR[:])
            # R^3*cos(3phi) = B(B^2-3A^2);  R^3*sin(3phi) = A(3B^2-A^2)
            T1 = S(0, FT)                                # r dead
            V._custom_dve(ops["cubic"], out=T1[:], in0=B, in1=Ap[:],
                          s0=3.0)
            T2p = S(2, FT)                               # = -R^3 sin(3phi)
            V._custom_dve(ops["cubic"], out=T2p[:], in0=Ap[:], in1=B,
                          s0=3.0)
            # cos(3phi - x0) = [T1*cx0 + T2p*sx0n] * iR^3
            cx0 = S(8, FT)
            V.tensor_scalar(out=cx0[:], in0=sh2[:], scalar1=-2.0,
                            scalar2=1.0, op0=Op.mult, op1=Op.add)
            w = S(1, FT)                                 # Ap dead
            vmul(w[:], T1[:], cx0[:])
            vv = S(3, FT)                                # B dead
            vmul(vv[:], T2p[:], sx0n[:])
            uu = S(0, FT)
            vadd(uu[:], w[:], vv[:])
            gq = S(1, FT)
            vmul(gq[:], uu[:], iR3[:])
            # K*(1 + cos) = K + K*cos: accumulate K*cos and K separately
            # (the K sums ride the idle ACT engine, saving a DVE pass)
            e = S(2, FT)
            vmul(e[:], gq[:], K[:])
            if defer_accum:
                return (e, ti * QT * 4 + 2)
            accumulate(e, ti * QT * 4 + 2, QT, Ft)
            return None

        # order: a small bond tile first primes the pipeline (least DMA
        # before first compute); the other bond tile LAST gives the kernel
        # a short serial tail instead of torsion's deep one.
        bond(0)
        deferred = None
        for ti in range(max(NTA, NTT)):
            # defer each angle tile's ACT accums until after the following
            # torsion body so torsion's critical Sqrts aren't queued
            # behind them on the in-order scalar engine
            da = angle(ti, defer_accum=True) if ti < NTA else None
            if ti < NTT:
                deferred = torsion(ti, defer_accum=(ti == NTT - 1))
            if da is not None:
                accumulate(da[0], da[1], QA, Fa)
        # the last torsion tile's ACT accums are emitted after the tail
        # bond so they overlap its DVE work instead of serializing ACT
        bond(1, tail=True)
        if deferred is not None:
            accumulate(deferred[0], deferred[1], QT, Ft)

        ones = pers.tile([P, 1], f32)
        V.memset(ones[:], 1.0)
        ps = psum.tile([1, PP * 4], f32)
        nc.tensor.matmul(out=ps[:], lhsT=ones[:], rhs=partials[:],
                         start=True, stop=True)
        psc = pers.tile([1, PP * 4], f32)
        V.tensor_copy(out=psc[:], in_=ps[:])
        s8 = pers.tile([1, PP], f32)
        V.tensor_tensor(out=s8[:], in0=psc[0:1, 0:PP * 4:4],
                        in1=psc[0:1, 1:PP * 4:4], op=Op.add)
        V.tensor_tensor(out=s8[:], in0=s8[:], in1=psc[0:1, 2:PP * 4:4],
                        op=Op.add)
        V.tensor_tensor(out=s8[:], in0=s8[:], in1=psc[0:1, 3:PP * 4:4],
                        op=Op.add)
        nc.sync.dma_start(out[:], s8[:])

    nc.compile()
    _BUILD_CACHE[key] = nc
    return nc


# ---------------------------------------------------------------------- main
def kernel(coords, global_params, bond_x0, angle_x0, tor_x0,
           bond_atoms, bond_param_idx, angle_atoms, angle_param_idx,
           tor_atoms, tor_param_idx, _trace=False):
    coords = np.asarray(coords, dtype=np.float32)
    K_table = np.asarray(global_params, dtype=np.float32)[:, 0]

    Fb, bidx, bKp, bx0p = _prep_type(np.asarray(bond_atoms),
                                     np.asarray(bond_param_idx),
                                     np.asarray(bond_x0, np.float32),
                                     K_table, 2)
    Fa, aidx, aKp, ax0p = _prep_type(np.asarray(angle_atoms),
                                     np.asarray(angle_param_idx),
                                     np.asarray(angle_x0, np.float32),
                                     K_table, 3)
    Ft, tidx, tKp, tx0p = _prep_type(np.asarray(tor_atoms),
                                     np.asarray(tor_param_idx),
                                     np.asarray(tor_x0, np.float32),
                                     K_table, 4)

    nc = _build(Fb, Fa, Ft)

    flat = coords.reshape(-1, 3).astype(ml_dtypes.bfloat16)

    in_maps = []
    bgq = _pack_g(flat, bidx, 2, QB, Fb)
    agq = _pack_g(flat, aidx, 3, QA, Fa)
    tgq = _pack_g(flat, tidx, 4, QT, Ft)
    bKq, bx0q = _pack_s(bKp, QB, Fb), _pack_s(bx0p, QB, Fb)
    aKq, ax0q = _pack_s(aKp, QA, Fa), _pack_s(ax0p, QA, Fa)
    tKq, tx0q = _pack_s(tKp, QT, Ft), _pack_s(tx0p, QT, Ft)

    for c in range(N_CORES):
        in_maps.append({
            "bg": bgq[c], "bK": bKq[c], "bx0": bx0q[c],
            "ag": agq[c], "aK": aKq[c], "ax0": ax0q[c],
            "tg": tgq[c], "tK": tKq[c], "tx0": tx0q[c],
        })

    from concourse.bass_utils import run_bass_kernel_spmd
    res = run_bass_kernel_spmd(nc, in_maps, list(range(N_CORES)),
                               trace=_trace)
    out = np.concatenate([res.results[c]["out"][0] for c in range(N_CORES)])
    if _trace:
        kernel._last_result = res
    return out.astype(np.float32)


# revision 34
# speedup vs baseline: 1.1882x; 1.0077x over previous
"""CartBonded whole-pose scoring on 8 Trainium2 NeuronCores.

Sharding (pose-major, per sharding hint): core c owns poses [8c, 8c+8).
Host: buckets term lists by pose (stable sort), pads each (pose, type)
bucket to fixed [128, F] tiles, expands per-term spring constants
K = global_params[param_idx], and materializes per-term atom coords in
tile layout (the multi-index indirect-DMA path mis-orders indices on
TRN2 HW; per-element descriptor gather is far below HBM rate anyway, so
the gather rides the same host permutation that shards the term lists).
All device inputs ship as bf16 in component-planar layout [P, 3, F]
with Q poses packed per tile, halving HBM traffic and enabling the
DVE's 2x bf16 perf mode (contiguous step-1, 4B-aligned slices).
Device: DVE does the tensor-tensor math in bf16 2x mode; the scalar
engine (ACT) takes all unary work (squares, sqrt, trig, dtype casts)
plus the per-pose segment sums via activation accum_out, so the two
engines run in parallel. GPSIMD does no elementwise work: it shares
SBUF ports with the DVE and measured 4-8x DVE slowdowns when active.
Torsion uses the triple-product identity sin-term = -|b2|*(n1.b3) to
skip the m1 = n1 x b2n cross entirely, and fused custom DVE cubics
B(B^2-3A^2), A(3B^2-A^2) for the triple angle. Angle uses
theta = pi/2 - atan(x/y) (y > 0 always, so no quadrant fixup; the ACT
Arctan table saturates correctly for huge x/y).
Final cross-partition reduce via a ones-vector matmul on PE.
"""

import numpy as np
import ml_dtypes

N_POSES = 64
MAX_ATOMS = 16384
N_CORES = 8
PP = N_POSES // N_CORES  # poses per core
P = 128
PI = float(np.pi)

QB, QA, QT = 4, 4, 4  # poses per device tile, by type

_BUILD_CACHE = {}
_DVE_OPS = None


# ------------------------------------------------------------- custom DVE ops
def _register_dve_ops():
    global _DVE_OPS
    if _DVE_OPS is not None:
        return _DVE_OPS
    import concourse.dve_ops as dve_ops
    from concourse.dve_ops import DveOp, OPS, get_dve_sub_opcode
    from concourse.dve_spec import Spec, Src0, Src1, C0, C1, C2, sq, select, \
        lower, _has_src1
    from concourse.dve_uop import DveOpSpec

    def reg(name, spec):
        for existing in OPS:
            if existing.name == name:
                return existing
        op = DveOp(name, spec, subdim=False, uops_sha={})
        OPS.append(op)
        dve_ops._SUB_OPCODE_FOR_NAME[name] = (
            dve_ops._CUSTOM_DVE_ROW_BASE + len(OPS) - 1)
        for ver in ("v3", "v4"):
            c = DveOpSpec(name=name, opcode=get_dve_sub_opcode(name),
                          uops=lower(spec, ver=ver), rd1_en=_has_src1(spec))
            op.uops_sha[ver] = c.sha(ver)
        return op

    f32 = np.float32
    sqsum = reg("ANT_CB_SQSUM", Spec(
        body=sq(Src0) + sq(Src1) + C0,
        reference=lambda in0, in1, s0, s1, imm2:
            in0.astype(f32) ** 2 + in1.astype(f32) ** 2 + s0,
    ))
    cubic = reg("ANT_CB_CUBIC", Spec(
        body=Src0 * (sq(Src0) - C0 * sq(Src1)),
        reference=lambda in0, in1, s0, s1, imm2:
            in0.astype(f32) * (in0.astype(f32) ** 2
                               - s0 * in1.astype(f32) ** 2),
    ))
    _t = C0 - Src0
    rangew = reg("ANT_CB_RANGEW", Spec(
        body=select(_t < C1, _t + C2, _t),
        reference=lambda in0, in1, s0, s1, imm2:
            np.where((s0 - in0.astype(f32)) < s1,
                     (s0 - in0.astype(f32)) + imm2, s0 - in0.astype(f32)),
    ))
    _DVE_OPS = dict(sqsum=sqsum, cubic=cubic, rangew=rangew)
    return _DVE_OPS


# ----------------------------------------------------------------- host prep
def _prep_type(atoms, param_idx, x0, K_table, arity):
    """Bucket terms by pose, pad to [P, F] tiles.

    Returns F, idx [N_POSES, arity, P, F] int64 (global flat atom row),
    K [N_POSES, P, F] f32 (0 on pads), x0 [N_POSES, P, F] f32.
    """
    n = atoms.shape[0]
    pose = (atoms[:, 0] // MAX_ATOMS).astype(np.int64)
    order = np.argsort(pose, kind="stable")
    pose_s = pose[order]
    atoms_s = atoms[order].astype(np.int64)
    x0_s = x0[order]
    K_s = K_table[param_idx[order]]

    counts = np.bincount(pose, minlength=N_POSES)
    F = -(-int(counts.max()) // P)  # ceil(max/P)
    F = -(-F // 8) * 8  # multiple of 8
    starts = np.zeros(N_POSES + 1, np.int64)
    np.cumsum(counts, out=starts[1:])
    r = np.arange(n, dtype=np.int64) - starts[pose_s]
    part = (r // F).astype(np.int64)
    free = (r % F).astype(np.int64)
    assert part.max() < P

    idx = np.zeros((N_POSES, arity, P, F), np.int64)
    idx[pose_s, :, part, free] = atoms_s
    Kp = np.zeros((N_POSES, P, F), np.float32)
    Kp[pose_s, part, free] = K_s
    x0p = np.zeros((N_POSES, P, F), np.float32)
    x0p[pose_s, part, free] = x0_s
    return F, idx, Kp, x0p


def _pack_g(flat_bf16, idx, arity, Q, F):
    """Gather + reorder to [N_CORES, PP//Q, arity, P, 3, Q*F] bf16."""
    g = flat_bf16[idx]  # [N_POSES, arity, P, F, 3]
    g = g.reshape(N_CORES, PP // Q, Q, arity, P, F, 3)
    g = np.ascontiguousarray(g.transpose(0, 1, 3, 4, 6, 2, 5))
    return g.reshape(N_CORES, PP // Q, arity, P, 3, Q * F)


def _pack_s(a, Q, F):
    """[N_POSES, P, F] f32 -> [N_CORES, PP//Q, P, Q*F] bf16."""
    a = a.reshape(N_CORES, PP // Q, Q, P, F)
    a = np.ascontiguousarray(a.transpose(0, 1, 3, 2, 4))
    return a.reshape(N_CORES, PP // Q, P, Q * F).astype(ml_dtypes.bfloat16)


# --------------------------------------------------------------- device build
def _build(Fb, Fa, Ft):
    key = (Fb, Fa, Ft)
    if key in _BUILD_CACHE:
        return _BUILD_CACHE[key]

    ops = _register_dve_ops()

    import concourse.tile as tile
    from concourse import bacc, mybir

    dt = mybir.dt
    f32, bf16 = dt.float32, dt.bfloat16
    Act = mybir.ActivationFunctionType
    Op = mybir.AluOpType

    nc = bacc.Bacc("TRN2", target_bir_lowering=False, debug=False,
                   num_devices=N_CORES)

    NTB, NTA, NTT = PP // QB, PP // QA, PP // QT
    FB, FA, FT = QB * Fb, QA * Fa, QT * Ft

    bg = nc.dram_tensor("bg", [NTB, 2, P, 3, FB], bf16,
                        kind="ExternalInput").ap()
    bK = nc.dram_tensor("bK", [NTB, P, FB], bf16, kind="ExternalInput").ap()
    bx0 = nc.dram_tensor("bx0", [NTB, P, FB], bf16, kind="ExternalInput").ap()
    ag = nc.dram_tensor("ag", [NTA, 3, P, 3, FA], bf16,
                        kind="ExternalInput").ap()
    aK = nc.dram_tensor("aK", [NTA, P, FA], bf16, kind="ExternalInput").ap()
    ax0 = nc.dram_tensor("ax0", [NTA, P, FA], bf16, kind="ExternalInput").ap()
    tg = nc.dram_tensor("tg", [NTT, 4, P, 3, FT], bf16,
                        kind="ExternalInput").ap()
    tK = nc.dram_tensor("tK", [NTT, P, FT], bf16, kind="ExternalInput").ap()
    tx0 = nc.dram_tensor("tx0", [NTT, P, FT], bf16, kind="ExternalInput").ap()
    out = nc.dram_tensor("out", [1, PP], f32, kind="ExternalOutput").ap()

    for v in (0.0, 1e-12, -PI, -PI / 2):
        t = nc.alloc_sbuf_tensor(f"constf32-{v}", [P, 1], f32)
        nc.gpsimd.memset(t.ap(), v)
        nc.const_aps.aps[(f32, v)] = t.ap()
    nc.all_engine_barrier()

    from contextlib import ExitStack

    with tile.TileContext(nc) as tc, ExitStack() as ctx:
        pers = ctx.enter_context(tc.tile_pool(name="pers", bufs=1))
        gpool = ctx.enter_context(tc.tile_pool(name="g", bufs=2))
        xkpool = ctx.enter_context(tc.tile_pool(name="xk", bufs=2))
        tp = ctx.enter_context(tc.tile_pool(name="tmp", bufs=1))
        psum = ctx.enter_context(tc.tile_pool(name="ps", bufs=1, space="PSUM"))

        # partials column layout: pose*4 + {0: bond, 1: angle, 2: torsion
        # K*cos term, 3: torsion K sum (the "+1" in K*(1+cos))}
        partials = pers.tile([P, PP * 4], f32)

        V = nc.vector
        A = nc.scalar

        def gather(g_dram, ti, slot, F):
            g = gpool.tile([P, 3, F], bf16, tag=f"g{slot}", name=f"g{slot}")
            nc.sync.dma_start(g[:], g_dram[ti, slot])
            return g

        def loadxk(K_dram, x0_dram, ti, F):
            K = xkpool.tile([P, F], bf16, tag="K", name="Kt")
            nc.sync.dma_start(K[:], K_dram[ti])
            X0 = xkpool.tile([P, F], bf16, tag="X0", name="X0t")
            nc.sync.dma_start(X0[:], x0_dram[ti])
            return K, X0

        # shared scratch tags (sized by the largest type so all reuse SBUF)
        def V3(i, F):
            return tp.tile([P, 3, F], bf16, tag=f"v3_{i}", name=f"v3_{i}")

        def S(i, F):
            return tp.tile([P, F], bf16, tag=f"s_{i}", name=f"s_{i}")

        def W(i, F):
            return tp.tile([P, F], f32, tag=f"w_{i}", name=f"w_{i}")

        def vsub(o, a, b):
            V.tensor_tensor(out=o, in0=a, in1=b, op=Op.subtract)

        def vmul(o, a, b):
            V.tensor_tensor(out=o, in0=a, in1=b, op=Op.mult)

        def vadd(o, a, b):
            V.tensor_tensor(out=o, in0=a, in1=b, op=Op.add)

        def vdiff(i, gA, gB):
            o = V3(i, gA.shape[2])
            vsub(o[:], gA[:], gB[:])
            return o

        def cross(dst_i, u, v, F):
            o = V3(dst_i, F)
            for c in range(3):
                ta = S(8, F)
                vmul(ta[:], u[:, (c + 1) % 3, :], v[:, (c + 2) % 3, :])
                tb = S(9, F)
                vmul(tb[:], u[:, (c + 2) % 3, :], v[:, (c + 1) % 3, :])
                vsub(o[:, c, :], ta[:], tb[:])
            return o

        def DM(F):
            # dot/norm2 plane scratch; bufs=2 so consecutive dots rotate
            return tp.tile([P, 3, F], bf16, tag="dm", name="dm", bufs=2)

        def QS(F):
            return tp.tile([P, 3, F], bf16, tag="qs", name="qs", bufs=2)

        def fold3(m, dst_i, F):
            acc = S(dst_i, F)
            vadd(acc[:], m[:, 0, :], m[:, 1, :])
            vadd(acc[:], acc[:], m[:, 2, :])
            return acc[:]

        def dot(u, v, F, dst_i):
            m = DM(F)
            vmul(m[:], u[:], v[:])
            return fold3(m, dst_i, F)

        def norm2(u, F, dst_i):
            # squares on ACT (one pass over all 3 planes), folds on DVE
            qs = QS(F)
            A.activation(qs[:], u[:], Act.Square)
            return fold3(qs, dst_i, F)

        def accumulate(e, base_col, Q, Fq):
            # per-pose segment sums on ACT via activation accum_out
            for q in range(Q):
                sl = slice(q * Fq, (q + 1) * Fq)
                scr = S(10, Fq)
                col = base_col + 4 * q
                A.activation(scr[:], e[:, sl], Act.Identity,
                             accum_out=partials[:, col:col + 1])

        def norm2_dve(m_i, dst_i, u, F):
            # all-DVE variant for the kernel head, before ACT warms up
            m = V3(m_i, F)
            vmul(m[:], u[:], u[:])
            o = S(dst_i, F)
            vadd(o[:], m[:, 0, :], m[:, 1, :])
            vadd(o[:], o[:], m[:, 2, :])
            return o

        # ---------------- bond: K * (|r0 - r1| - x0)^2
        def bond(ti, tail=False):
            g0 = gather(bg, ti, 0, FB)
            g1 = gather(bg, ti, 1, FB)
            K, X0 = loadxk(bK, bx0, ti, FB)
            d = vdiff(0, g0, g1)
            D2 = norm2_dve(1, 0, d, FB)                  # s0
            dist = S(1, FB)
            A.activation(dist[:], D2[:], Act.Sqrt, bias=1e-12)
            dd = S(5, FB)
            vsub(dd[:], dist[:], X0[:])
            sq = S(3, FB)
            if tail:
                vmul(sq[:], dd[:], dd[:])
            else:
                A.activation(sq[:], dd[:], Act.Square)
            if tail:
                # last tile: segment sums on the (idle-by-now) DVE so the
                # kernel tail is not a serial ACT chain
                for q in range(QB):
                    sl = slice(q * Fb, (q + 1) * Fb)
                    e = S(4, Fb)
                    col = ti * QB * 4 + 0 + 4 * q
                    V.scalar_tensor_tensor(
                        out=e[:], in0=sq[:, sl], scalar=0.0, in1=K[:, sl],
                        op0=Op.add, op1=Op.mult,
                        accum_out=partials[:, col:col + 1])
            else:
                e = S(4, FB)
                vmul(e[:], sq[:], K[:])
                accumulate(e, ti * QB * 4 + 0, QB, Fb)

        # ------- angle: K * (theta - x0)^2, theta = pi/2 - atan(x/y), y>0
        # |uxv|^2 = |u|^2*|v|^2 - (u.v)^2 (Lagrange), clamped at 0 --
        # cheaper than the explicit cross product; the rare near-parallel
        # cancellation error is noise at the pose-sum level.
        def angle(ti):
            g0 = gather(ag, ti, 0, FA)
            g1 = gather(ag, ti, 1, FA)
            g2 = gather(ag, ti, 2, FA)
            K, X0 = loadxk(aK, ax0, ti, FA)
            xc = S(7, FA)
            A.activation(xc[:], X0[:], Act.Identity, bias=-PI / 2)
            u = vdiff(0, g0, g1)
            v = vdiff(1, g2, g1)
            # issue both squares on ACT before the x-dot occupies the DVE,
            # so the uu/vv folds find their inputs ready
            qs_u = QS(FA)
            A.activation(qs_u[:], u[:], Act.Square)
            qs_v = QS(FA)
            A.activation(qs_v[:], v[:], Act.Square)
            x = dot(u, v, FA, 0)
            uu = fold3(qs_u, 1, FA)
            vv = fold3(qs_v, 2, FA)
            x2 = S(3, FA)
            vmul(x2[:], x, x)
            m = S(4, FA)
            vmul(m[:], uu, vv)
            Sn = S(5, FA)
            vsub(Sn[:], m[:], x2[:])
            Sc = S(6, FA)
            A.activation(Sc[:], Sn[:], Act.Relu)
            y = W(0, FA)
            A.activation(y[:], Sc[:], Act.Sqrt, bias=1e-12)
            yi = W(1, FA)
            V.reciprocal_approx_fast(yi[:], y[:])
            t = S(2, FA)
            vmul(t[:], x, yi[:])
            phi = S(3, FA)
            A.activation(phi[:], t[:], Act.Arctan)
            dd = S(4, FA)
            vadd(dd[:], phi[:], xc[:])
            sq = S(5, FA)
            A.activation(sq[:], dd[:], Act.Square)
            e = S(6, FA)
            vmul(e[:], sq[:], K[:])
            accumulate(e, ti * QA * 4 + 1, QA, Fa)

        # ---------------- torsion: K * (1 + cos(3*phi - x0))
        def torsion(ti, defer_accum=False):
            g0 = gather(tg, ti, 0, FT)
            g1 = gather(tg, ti, 1, FT)
            g2 = gather(tg, ti, 2, FT)
            g3 = gather(tg, ti, 3, FT)
            K, X0 = loadxk(tK, tx0, ti, FT)
            # K-sums depend only on the K DMA: accumulate right away so
            # they never sit on the kernel tail
            accumulate(K, ti * QT * 4 + 3, QT, Ft)
            # x0 trig first: only needs X0, keeps ACT busy during crosses.
            # cos(x0) = 1 - 2*sin(x0/2)^2 keeps the Sin arg inside [0, pi);
            # the cheap DVE combine is emitted LATE so the in-order DVE
            # queue never waits on this ACT chain.
            sh = S(5, FT)
            A.activation(sh[:], X0[:], Act.Sin, scale=0.5)
            sh2 = S(6, FT)
            A.activation(sh2[:], sh[:], Act.Square)
            sx0n = S(7, FT)
            A.activation(sx0n[:], X0[:], Act.Sin, bias=-PI)
            b1 = vdiff(0, g1, g0)
            b2 = vdiff(1, g2, g1)
            b3 = vdiff(2, g3, g2)
            n1 = cross(3, b1, b2, FT)                    # b1 dead, v3_0 free
            n2 = cross(4, b2, b3, FT)
            S2 = norm2(b2, FT, 1)
            r = S(0, FT)
            A.activation(r[:], S2, Act.Sqrt, bias=1e-12)
            d13 = dot(n1, b3, FT, 2)
            B = dot(n1, n2, FT, 3)
            # ref sin-term A = (n1 x b2_hat).n2 = -|b2|*(n1.b3); carry
            # Ap = -A and flip the sin sign downstream.
            Ap = S(1, FT)
            vmul(Ap[:], r[:], d13)
            R2 = W(0, FT)
            V._custom_dve(ops["sqsum"], out=R2[:], in0=Ap[:], in1=B,
                          s0=1e-6)
            r2i = W(1, FT)
            V.reciprocal_approx_fast(r2i[:], R2[:])
            iR = S(2, FT)                                # d13 dead
            A.activation(iR[:], r2i[:], Act.Sqrt)
            iR3 = S(4, FT)
            vmul(iR3[:], r2i[:], iR[:])
            # R^3*cos(3phi) = B(B^2-3A^2);  R^3*sin(3phi) = A(3B^2-A^2)
            T1 = S(0, FT)                                # r dead
            V._custom_dve(ops["cubic"], out=T1[:], in0=B, in1=Ap[:],
                          s0=3.0)
            T2p = S(2, FT)                               # = -R^3 sin(3phi)
            V._custom_dve(ops["cubic"], out=T2p[:], in0=Ap[:], in1=B,
                          s0=3.0)
            # cos(3phi - x0) = [T1*cx0 + T2p*sx0n] * iR^3
            cx0 = S(8, FT)
            V.tensor_scalar(out=cx0[:], in0=sh2[:], scalar1=-2.0,
                            scalar2=1.0, op0=Op.mult, op1=Op.add)
            w = S(1, FT)                                 # Ap dead
            vmul(w[:], T1[:], cx0[:])
            vv = S(3, FT)                                # B dead
            vmul(vv[:], T2p[:], sx0n[:])
            uu = S(0, FT)
            vadd(uu[:], w[:], vv[:])
            gq = S(1, FT)
            vmul(gq[:], uu[:], iR3[:])
            # K*(1 + cos) = K + K*cos: accumulate K*cos and K separately
            # (the K sums ride the idle ACT engine, saving a DVE pass)
            e = S(2, FT)
            vmul(e[:], gq[:], K[:])
            if defer_accum:
                return (e, ti * QT * 4 + 2)
            accumulate(e, ti * QT * 4 + 2, QT, Ft)
            return None

        # order: a small bond tile first primes the pipeline (least DMA
        # before first compute); the other bond tile LAST gives the kernel
        # a short serial tail instead of torsion's deep one.
        bond(0)
        deferred = None
        for ti in range(max(NTA, NTT)):
            if ti < NTA:
                angle(ti)
            if ti < NTT:
                deferred = torsion(ti, defer_accum=(ti == NTT - 1))
        # the last torsion tile's ACT accums are emitted after the tail
        # bond so they overlap its DVE work instead of serializing ACT
        bond(1, tail=True)
        if deferred is not None:
            accumulate(deferred[0], deferred[1], QT, Ft)

        ones = pers.tile([P, 1], f32)
        V.memset(ones[:], 1.0)
        ps = psum.tile([1, PP * 4], f32)
        nc.tensor.matmul(out=ps[:], lhsT=ones[:], rhs=partials[:],
                         start=True, stop=True)
        psc = pers.tile([1, PP * 4], f32)
        V.tensor_copy(out=psc[:], in_=ps[:])
        s8 = pers.tile([1, PP], f32)
        V.tensor_tensor(out=s8[:], in0=psc[0:1, 0:PP * 4:4],
                        in1=psc[0:1, 1:PP * 4:4], op=Op.add)
        V.tensor_tensor(out=s8[:], in0=s8[:], in1=psc[0:1, 2:PP * 4:4],
                        op=Op.add)
        V.tensor_tensor(out=s8[:], in0=s8[:], in1=psc[0:1, 3:PP * 4:4],
                        op=Op.add)
        nc.sync.dma_start(out[:], s8[:])

    nc.compile()
    _BUILD_CACHE[key] = nc
    return nc


# ---------------------------------------------------------------------- main
def kernel(coords, global_params, bond_x0, angle_x0, tor_x0,
           bond_atoms, bond_param_idx, angle_atoms, angle_param_idx,
           tor_atoms, tor_param_idx, _trace=False):
    coords = np.asarray(coords, dtype=np.float32)
    K_table = np.asarray(global_params, dtype=np.float32)[:, 0]

    Fb, bidx, bKp, bx0p = _prep_type(np.asarray(bond_atoms),
                                     np.asarray(bond_param_idx),
                                     np.asarray(bond_x0, np.float32),
                                     K_table, 2)
    Fa, aidx, aKp, ax0p = _prep_type(np.asarray(angle_atoms),
                                     np.asarray(angle_param_idx),
                                     np.asarray(angle_x0, np.float32),
                                     K_table, 3)
    Ft, tidx, tKp, tx0p = _prep_type(np.asarray(tor_atoms),
                                     np.asarray(tor_param_idx),
                                     np.asarray(tor_x0, np.float32),
                                     K_table, 4)

    nc = _build(Fb, Fa, Ft)

    flat = coords.reshape(-1, 3).astype(ml_dtypes.bfloat16)

    in_maps = []
    bgq = _pack_g(flat, bidx, 2, QB, Fb)
    agq = _pack_g(flat, aidx, 3, QA, Fa)
    tgq = _pack_g(flat, tidx, 4, QT, Ft)
    bKq, bx0q = _pack_s(bKp, QB, Fb), _pack_s(bx0p, QB, Fb)
    aKq, ax0q = _pack_s(aKp, QA, Fa), _pack_s(ax0p, QA, Fa)
    tKq, tx0q = _pack_s(tKp, QT, Ft), _pack_s(tx0p, QT, Ft)

    for c in range(N_CORES):
        in_maps.append({
            "bg": bgq[c], "bK": bKq[c], "bx0": bx0q[c],
            "ag": agq[c], "aK": aKq[c], "ax0": ax0q[c],
            "tg": tgq[c], "tK": tKq[c], "tx0": tx0q[c],
        })

    from concourse.bass_utils import run_bass_kernel_spmd
    res = run_bass_kernel_spmd(nc, in_maps, list(range(N_CORES)),
                               trace=_trace)
    out = np.concatenate([res.results[c]["out"][0] for c in range(N_CORES)])
    if _trace:
        kernel._last_result = res
    return out.astype(np.float32)
